# revision 3
# baseline (speedup 1.0000x reference)
"""Multi-head attention (B=2, S=1024, D=768, H=12) on 8 TRN2 NeuronCores. v8.

Sharding: batch x head-group. Core c handles batch b = c // 4 and heads
3*(c%4) .. 3*(c%4)+2: q/k/v projections for its heads, attention, partial
output projection through its rows of Wo. Host sums 4 partials per batch.

v4: every PSUM intermediate is processed at 512-column granularity so each
tile is exactly one PSUM bank (2 KB/partition). That allows 8 concurrently
live PSUM buffers (3 logits-halves + 4 ctx-halves + 1 small) instead of 4
double-banked ones, which removes the buffer-rotation serialization that
dominated v2/v3: logits->mask-add->exp->AV for consecutive chunks now
overlap across PE/DVE/ACT.

Other structure (from v2/v3):
- Key compaction to the exact valid-key count (padded to 128 for the
  chunked loops, DMA-trimmed to n_trim); NEFF cached per (nkc, n_trim).
- fp16 inputs/weights/intermediates, fp8 e4m3 additive mask added into
  PSUM in place by the DVE, exp on ACT straight from PSUM with the
  key-padding/-8 bias per partition (the -8 shift guards fp16 exp
  overflow; it cancels in softmax).
- Softmax denominators ride the AV matmul as a ones-column of v.
- Copies balanced across ACT and DVE.
"""

import numpy as np

B, SQ, D, H = 2, 1024, 768, 12
DH = D // H            # 64
HPC = 3                # heads per core
N_CORES = 8
GPB = 4                # head-groups (cores) per batch
KT = 6                 # k-tiles over the contraction dim 768
NEG = -1.0e30
ESH = -8.0             # exp shift
HF = 512               # half width

_CACHE = {}


def _build(nkc, n_trim, repeats=1, stage=3, loop_n=0):
    import concourse.tile as tile
    import concourse.mybir as mybir
    from concourse import bacc

    SK_P = nkc * 128
    assert 0 < n_trim <= SK_P
    f32 = mybir.dt.float32
    f16 = mybir.dt.float16
    fp8 = mybir.dt.float8e4
    AF = mybir.ActivationFunctionType

    nc = bacc.Bacc("TRN2", target_bir_lowering=False, debug=False,
                   num_devices=N_CORES)

    nfull = n_trim // 128
    r_t = n_trim - 128 * nfull
    qT = nc.dram_tensor("qT", [128, KT * SQ], f16, kind="ExternalInput").ap()
    kT = nc.dram_tensor("kT", [128, KT * n_trim], f16,
                        kind="ExternalInput").ap()
    vT = nc.dram_tensor("vT", [128, KT * n_trim], f16,
                        kind="ExternalInput").ap()
    Wqk = nc.dram_tensor("Wqk", [D, 384], f16, kind="ExternalInput").ap()
    WvA = nc.dram_tensor("WvA", [D, 65 * HPC], f16, kind="ExternalInput").ap()
    WoR = nc.dram_tensor("WoR", [HPC, DH, D], f16, kind="ExternalInput").ap()
    padc = nc.dram_tensor("padc", [128, nkc], f32, kind="ExternalInput").ap()
    ones64 = nc.dram_tensor("ones64", [1, DH], f16, kind="ExternalInput").ap()
    maskT = nc.dram_tensor("maskT", [128, max(nfull, 1) * HPC * SQ], fp8,
                           kind="ExternalInput").ap()
    maskU = nc.dram_tensor("maskU", [128, HPC * SQ], fp8,
                           kind="ExternalInput").ap()
    out_d = nc.dram_tensor("out", [SQ, D], f16, kind="ExternalOutput").ap()

    with tile.TileContext(nc) as tc:
        with (
            tc.tile_pool(name="consts", bufs=1) as cp,
            tc.tile_pool(name="xt", bufs=2) as xtp,
            tc.tile_pool(name="qk", bufs=2) as qkp,
            tc.tile_pool(name="vv", bufs=2) as vvp,
            tc.tile_pool(name="mask", bufs=2) as mkp,
            tc.tile_pool(name="pt", bufs=2 * nkc) as ptp,
            tc.tile_pool(name="norm", bufs=2) as nmp,
            tc.tile_pool(name="tmp", bufs=4) as tmp,
            tc.tile_pool(name="outs", bufs=2) as otp,
            tc.tile_pool(name="psS", bufs=3, space="PSUM") as psS,
            tc.tile_pool(name="psC", bufs=4, space="PSUM") as psC,
            tc.tile_pool(name="psR", bufs=1, space="PSUM") as psR,
        ):
            # ---- constants (amortized across repeats) ----
            wq = []
            for t in range(KT):
                w1 = cp.tile([128, 384], f16, tag=f"wq{t}")
                nc.sync.dma_start(w1[:], Wqk[t * 128:(t + 1) * 128, :])
                wq.append(w1)
            pad = cp.tile([128, nkc], f32, tag="pad")
            nc.sync.dma_start(pad[:], padc)
            wv = []
            for t in range(KT):
                w3 = cp.tile([128, 65 * HPC], f16, tag=f"wv{t}")
                nc.sync.dma_start(w3[:], WvA[t * 128:(t + 1) * 128, :])
                wv.append(w3)
            o64 = cp.tile([1, DH], f16, tag="o64")
            nc.sync.dma_start(o64[:], ones64)
            wo01 = cp.tile([128, D], f16, tag="wo01")
            nc.sync.dma_start(wo01[:], WoR[0:2].rearrange("a b c -> (a b) c"))
            wo2 = cp.tile([DH, D], f16, tag="wo2")
            nc.sync.dma_start(wo2[:], WoR[2])

            def load_x(x_dram, nl, tag):
                xt_t = xtp.tile([128, KT * nl], f16, tag=tag)
                nc.sync.dma_start(xt_t[:], x_dram[:, 0:KT * nl])
                return xt_t

            def nchunks(n):
                return [(i, min(HF, n - i)) for i in range(0, n, HF)]

            # ---- q^T / k^T projection, one [rows, n] chunk, half-wise ----
            def proj_one(xbig, xs, col0, rows, n, tag, nl=None, eng=None):
                nl = n if nl is None else nl
                dst = qkp.tile([rows, n], f16, tag=tag)
                for n0, nw in nchunks(nl):
                    pps = psS.tile([128, HF], f32, tag="sps")
                    for t in range(KT):
                        nc.tensor.matmul(
                            pps[0:rows, 0:nw],
                            wq[t][:, col0: col0 + rows],
                            xbig[:, t * xs + n0:t * xs + n0 + nw],
                            start=(t == 0), stop=(t == KT - 1))
                    if eng == "dve":
                        nc.vector.tensor_copy(dst[:, n0:n0 + nw],
                                              pps[0:rows, 0:nw])
                    else:
                        nc.scalar.copy(dst[:, n0:n0 + nw], pps[0:rows, 0:nw])
                if nl < n:
                    nc.gpsimd.memset(dst[:, nl:n], 0.0)
                return dst

            def load_mask():
                mb = mkp.tile([128, max(nfull, 1) * HPC * SQ], fp8,
                              tag="mask")
                nc.sync.dma_start(mb[:, 0:nfull * HPC * SQ],
                                  maskT[:, 0:nfull * HPC * SQ])
                mt = None
                if r_t:
                    mt = mkp.tile([128, HPC * SQ], fp8, tag="mtail")
                    nc.gpsimd.memset(mt[:], 0.0)
                    nc.sync.dma_start(mt[0:r_t, :], maskU[0:r_t, :])
                return mb, mt

            def load_rep():
                qx = load_x(qT, SQ, "xq")
                kx = load_x(kT, n_trim, "xk")
                mbig, mtail = load_mask()
                vx = load_x(vT, n_trim, "xv")
                return qx, kx, vx, mbig, mtail

            def emit_rep(tiles):
                qx, kx, vx, mbig, mtail = tiles
                if stage == 0:
                    for t in range(8):
                        ot = otp.tile([128, D], f16, tag=f"ot{t % 3}")
                        nc.gpsimd.memset(ot[:], 0.0)
                        nc.sync.dma_start(out_d[t * 128:(t + 1) * 128, :],
                                          ot[:])
                    return
                q_c0 = proj_one(qx, SQ, 0, 128, SQ, "q0")
                q_c1 = proj_one(qx, SQ, 128, DH, SQ, "q1", eng="dve")
                k_c0 = proj_one(kx, n_trim, 192, 128, SK_P, "k0", n_trim)
                k_c1 = proj_one(kx, n_trim, 192 + 128, DH, SK_P, "k1",
                                n_trim, eng="dve")
                qh = [q_c0[0:DH, :], q_c0[DH:128, :], q_c1]
                kh = [k_c0[0:DH, :], k_c0[DH:128, :], k_c1]

                # ---- attention ----
                vtiles = []
                cn01 = nmp.tile([128, SQ], f16, tag="cn01")
                cn2 = nmp.tile([DH, SQ], f16, tag="cn2")
                ctxh = {}
                pts = {}

                def emit_tile(j, i):
                    msrc = mtail if i >= nfull else mbig
                    mof = 0 if i >= nfull else i * HPC * SQ
                    pt = ptp.tile([128, SQ], f16, tag="pt")
                    for n0, nw in nchunks(SQ):
                        sps = psS.tile([128, HF], f32, tag="sps")
                        nc.tensor.matmul(
                            sps[:, 0:nw],
                            kh[j][:, i * 128:(i + 1) * 128],
                            qh[j][:, n0:n0 + nw],
                            start=True, stop=True)
                        nc.vector.tensor_add(
                            sps[:, 0:nw], sps[:, 0:nw],
                            msrc[:, mof + j * SQ + n0:mof + j * SQ + n0 + nw])
                        nc.scalar.activation(pt[:, n0:n0 + nw], sps[:, 0:nw],
                                             AF.Exp, bias=pad[:, i:i + 1],
                                             scale=1.0)
                    pts[(j, i)] = pt

                def emit_av(j, i):
                    pt_ = pts[(j, i)]
                    for hh, (n0, nw) in enumerate(nchunks(SQ)):
                        if (j, hh) not in ctxh:
                            ctxh[(j, hh)] = psC.tile([65, HF], f32, tag="cx",
                                                     name=f"cx{j}_{hh}")
                        nc.tensor.matmul(
                            ctxh[(j, hh)][:, 0:nw],
                            vtiles[i][:, j * 65:(j + 1) * 65],
                            pt_[:, n0:n0 + nw],
                            start=(i == 0), stop=(i == nkc - 1))
                    del pts[(j, i)]

                def emit_norm(j):
                    for hh, (n0, nw) in enumerate(nchunks(SQ)):
                        ctx = ctxh.pop((j, hh))
                        rd = tmp.tile([1, HF], f32, tag="rd")
                        nc.vector.reciprocal(rd[:, 0:nw],
                                             ctx[DH:DH + 1, 0:nw])
                        rdb = tmp.tile([DH, HF], f32, tag="rdb")
                        nc.gpsimd.partition_broadcast(rdb[:, 0:nw],
                                                      rd[:, 0:nw])
                        dst = (cn01[j * DH:(j + 1) * DH] if j < 2 else
                               cn2[:])
                        nc.vector.tensor_mul(dst[:, n0:n0 + nw],
                                             ctx[0:DH, 0:nw], rdb[:, 0:nw])

                def vproj_tile(i, vx):
                    rl = min(128, n_trim - i * 128)
                    vps = psR.tile([128, 65 * HPC], f32, tag="rr")
                    for t in range(KT):
                        nc.tensor.matmul(
                            vps[0:rl, :],
                            vx[:, t * n_trim + i * 128:
                               t * n_trim + i * 128 + rl],
                            wv[t][:],
                            start=(t == 0), stop=(t == KT - 1))
                    vt = vvp.tile([128, 65 * HPC], f16, tag=f"v{i}")
                    if rl < 128:
                        nc.gpsimd.memset(vt[:], 0.0)
                    nc.scalar.copy(vt[0:rl, :], vps[0:rl, :])
                    for j in range(HPC):
                        nc.gpsimd.memset(
                            vt[0:rl, j * 65 + DH:j * 65 + DH + 1], 1.0)
                    vtiles.append(vt)

                # head 0 logits chain (AVs deferred until v exists)
                for i in range(nkc):
                    emit_tile(0, i)

                # head 1 chain; per tile: v-proj(i) + AV(0,i) fused in
                for i in range(nkc):
                    vproj_tile(i, vx)
                    emit_av(0, i)
                    emit_tile(1, i)
                    if i > 0:
                        emit_av(1, i - 1)
                emit_norm(0)
                # head 2 chain
                for i in range(nkc):
                    emit_tile(2, i)
                    if i == 0:
                        emit_av(1, nkc - 1)
                    else:
                        emit_av(2, i - 1)
                emit_norm(1)
                emit_av(2, nkc - 1)
                emit_norm(2)

                # ---- output projection ----
                for t in range(8):
                    ot = otp.tile([128, D], f16, tag=f"ot{t % 3}")
                    for hh, (n0, nw) in enumerate(((0, 512), (512, 256))):
                        ops = psS.tile([128, HF], f32, tag="sps")
                        nc.tensor.matmul(
                            ops[:, 0:nw],
                            cn01[:, t * 128:(t + 1) * 128],
                            wo01[:, n0:n0 + nw],
                            start=True, stop=False)
                        nc.tensor.matmul(
                            ops[:, 0:nw],
                            cn2[:, t * 128:(t + 1) * 128],
                            wo2[:, n0:n0 + nw],
                            start=False, stop=True)
                        if (2 * t + hh) % 2 == 0:
                            nc.vector.tensor_copy(ot[:, n0:n0 + nw],
                                                  ops[:, 0:nw])
                        else:
                            nc.scalar.copy(ot[:, n0:n0 + nw], ops[:, 0:nw])
                    nc.gpsimd.dma_start(out_d[t * 128:(t + 1) * 128, :],
                                        ot[:])

            def emit_body():
                tiles = load_rep()
                for r in range(repeats):
                    nxt = load_rep() if r + 1 < repeats else None
                    emit_rep(tiles)
                    tiles = nxt

            if loop_n:
                ET = mybir.EngineType
                with tc.For_i(0, loop_n, 1,
                              hint_engines=(ET.PE, ET.DVE, ET.Activation,
                                            ET.Pool, ET.SP)):
                    emit_body()
            else:
                emit_body()

    nc.compile()
    return nc


def prep_inputs(value, key, query, key_padding_mask, attn_mask,
                Wq, Wk, Wv, Wo, bq, bk, bv, bo):
    import ml_dtypes

    f = np.float32
    h = np.float16
    value = np.asarray(value, f)
    key = np.asarray(key, f)
    query = np.asarray(query, f)
    key_padding_mask = np.asarray(key_padding_mask)
    attn_mask = np.asarray(attn_mask, f)
    Wq, Wk, Wv, Wo = (np.asarray(w, f) for w in (Wq, Wk, Wv, Wo))
    for b_ in (bq, bk, bv):
        assert np.abs(np.asarray(b_)).max() == 0.0, \
            "nonzero qkv biases unsupported in this build"

    scale = f(1.0 / np.sqrt(DH))
    idx = [np.nonzero(key_padding_mask[b])[0] for b in range(B)]
    n_trim = max(1, max(len(ix) for ix in idx))
    nkc = -(-n_trim // 128)
    SK_P = nkc * 128

    xT = {}
    padcs = {}
    for b in range(B):
        ix = idx[b]
        n = len(ix)
        kc = np.zeros((SK_P, D), f)
        vc = np.zeros((SK_P, D), f)
        kc[:n] = key[b][ix]
        vc[:n] = value[b][ix]
        def pack(m, nl):
            # [768, nl] -> [128, KT*nl], block t = rows t*128:(t+1)*128
            mm = m.T.astype(h)[:, :nl]
            return np.ascontiguousarray(
                mm.reshape(KT, 128, nl).transpose(1, 0, 2).reshape(
                    128, KT * nl))
        xT[("q", b)] = pack(query[b], SQ)
        xT[("k", b)] = pack(kc, n_trim)
        xT[("v", b)] = pack(vc, n_trim)
        pc = np.full((SK_P,), ESH, f)
        pc[n:] = NEG
        padcs[b] = np.ascontiguousarray(pc.reshape(nkc, 128).T)

    in_maps = []
    for c in range(N_CORES):
        b, g = divmod(c, GPB)
        h0 = g * HPC
        cols = slice(h0 * DH, (h0 + HPC) * DH)
        WqkA = np.zeros((D, 384), f)
        WqkA[:, 0:192] = Wq[:, cols] * scale
        WqkA[:, 192:384] = Wk[:, cols]
        WvC = np.zeros((D, 65 * HPC), f)
        for j in range(HPC):
            hc = slice((h0 + j) * DH, (h0 + j + 1) * DH)
            WvC[:, j * 65:j * 65 + DH] = Wv[:, hc]
        WoR = np.ascontiguousarray(Wo[cols].reshape(HPC, DH, D)).astype(h)
        ix = idx[b]
        mk = np.zeros((SK_P, HPC * SQ), f)
        for j in range(HPC):
            mk[:len(ix), j * SQ:(j + 1) * SQ] = attn_mask[b, h0 + j].T[ix]
        mk8 = mk.astype(ml_dtypes.float8_e4m3fn)
        nfull = n_trim // 128
        r_t = n_trim - 128 * nfull
        mbig = np.zeros((128, max(nfull, 1) * HPC * SQ),
                        ml_dtypes.float8_e4m3fn)
        for i in range(nfull):
            mbig[:, i * HPC * SQ:(i + 1) * HPC * SQ] = \
                mk8[i * 128:(i + 1) * 128]
        mtail = np.zeros((128, HPC * SQ), ml_dtypes.float8_e4m3fn)
        if r_t:
            mtail[0:r_t] = mk8[nfull * 128:nfull * 128 + r_t]
        in_maps.append({
            "qT": xT[("q", b)],
            "kT": xT[("k", b)],
            "vT": xT[("v", b)],
            "Wqk": WqkA.astype(h),
            "WvA": WvC.astype(h),
            "WoR": WoR,
            "maskT": mbig,
            "maskU": mtail,
            "padc": padcs[b],
            "ones64": np.ones((1, DH), h),
        })
    return in_maps, nkc, n_trim


def get_nc(nkc, n_trim, repeats=1, stage=3, loop_n=0):
    key = ("nc", nkc, n_trim, repeats, stage, loop_n)
    if key not in _CACHE:
        _CACHE[key] = _build(nkc, n_trim, repeats, stage, loop_n)
    return _CACHE[key]


def assemble(results, bo):
    out = np.zeros((B, SQ, D), np.float32)
    for c in range(N_CORES):
        out[c // GPB] += results[c]["out"].astype(np.float32)
    return out + np.asarray(bo, np.float32)


def bench_prep(inputs):
    import os
    in_maps, nkc, n_trim = prep_inputs(**inputs)
    stage = int(os.environ.get("STAGE", "3"))
    return in_maps, (lambda reps=1, loop_n=0: get_nc(nkc, n_trim, reps,
                                                     stage, loop_n))


def kernel(value, key, query, key_padding_mask, attn_mask,
           Wq, Wk, Wv, Wo, bq, bk, bv, bo, **extra):
    from concourse.bass_utils import run_bass_kernel_spmd

    in_maps, nkc, n_trim = prep_inputs(value, key, query, key_padding_mask,
                                       attn_mask, Wq, Wk, Wv, Wo,
                                       bq, bk, bv, bo)
    nc = get_nc(nkc, n_trim)
    res = run_bass_kernel_spmd(nc, in_maps, core_ids=list(range(N_CORES)))
    _CACHE["last_results"] = res
    return assemble(res.results, bo)


# revision 4
# speedup vs baseline: 1.0969x; 1.0969x over previous
"""Multi-head attention (B=2, S=1024, D=768, H=12) on 8 TRN2 NeuronCores. v9.

Sharding: batch x head-group. Core c handles batch b = c // 4 and heads
3*(c%4) .. 3*(c%4)+2: q/k/v projections for its heads, attention, partial
output projection through its rows of Wo. Host sums 4 partials per batch.

v4: every PSUM intermediate is processed at 512-column granularity so each
tile is exactly one PSUM bank (2 KB/partition). That allows 8 concurrently
live PSUM buffers (3 logits-halves + 4 ctx-halves + 1 small) instead of 4
double-banked ones, which removes the buffer-rotation serialization that
dominated v2/v3: logits->mask-add->exp->AV for consecutive chunks now
overlap across PE/DVE/ACT.

Other structure (from v2/v3):
- Key compaction to the exact valid-key count (padded to 128 for the
  chunked loops, DMA-trimmed to n_trim); NEFF cached per (nkc, n_trim).
- fp16 inputs/weights/intermediates, fp8 e4m3 additive mask added into
  PSUM in place by the DVE, exp on ACT straight from PSUM with the
  key-padding/-8 bias per partition (the -8 shift guards fp16 exp
  overflow; it cancels in softmax).
- Softmax denominators ride the AV matmul as a ones-column of v.
- Copies balanced across ACT and DVE.
"""

import numpy as np

B, SQ, D, H = 2, 1024, 768, 12
DH = D // H            # 64
HPC = 3                # heads per core
N_CORES = 8
GPB = 4                # head-groups (cores) per batch
KT = 6                 # k-tiles over the contraction dim 768
NEG = -1.0e30
ESH = -8.0             # exp shift
HF = 512               # half width

_CACHE = {}


def _build(nkc, n_trim, repeats=1, stage=3, loop_n=0):
    import concourse.tile as tile
    import concourse.mybir as mybir
    from concourse import bacc

    SK_P = nkc * 128
    assert 0 < n_trim <= SK_P
    f32 = mybir.dt.float32
    f16 = mybir.dt.float16
    fp8 = mybir.dt.float8e4
    AF = mybir.ActivationFunctionType

    nc = bacc.Bacc("TRN2", target_bir_lowering=False, debug=False,
                   num_devices=N_CORES)

    nfull = n_trim // 128
    r_t = n_trim - 128 * nfull
    qT = nc.dram_tensor("qT", [128, KT * SQ], f16, kind="ExternalInput").ap()
    kT = nc.dram_tensor("kT", [128, KT * n_trim], f16,
                        kind="ExternalInput").ap()
    vT = nc.dram_tensor("vT", [128, KT * n_trim], f16,
                        kind="ExternalInput").ap()
    Wqk = nc.dram_tensor("Wqk", [D, 384], f16, kind="ExternalInput").ap()
    WvA = nc.dram_tensor("WvA", [D, 65 * HPC], f16, kind="ExternalInput").ap()
    WoR = nc.dram_tensor("WoR", [HPC, DH, D], f16, kind="ExternalInput").ap()
    padc = nc.dram_tensor("padc", [128, nkc], f32, kind="ExternalInput").ap()
    ones64 = nc.dram_tensor("ones64", [1, DH], f16, kind="ExternalInput").ap()
    idf8 = nc.dram_tensor("idf8", [128, 128], fp8, kind="ExternalInput").ap()
    maskT = nc.dram_tensor("maskT", [128, max(nfull, 1) * HPC * SQ], fp8,
                           kind="ExternalInput").ap()
    maskU = nc.dram_tensor("maskU", [128, HPC * SQ], fp8,
                           kind="ExternalInput").ap()
    out_d = nc.dram_tensor("out", [SQ, D], f16, kind="ExternalOutput").ap()

    with tile.TileContext(nc) as tc:
        with (
            tc.tile_pool(name="consts", bufs=1) as cp,
            tc.tile_pool(name="xt", bufs=2) as xtp,
            tc.tile_pool(name="qk", bufs=2) as qkp,
            tc.tile_pool(name="vv", bufs=2) as vvp,
            tc.tile_pool(name="mask", bufs=2) as mkp,
            tc.tile_pool(name="pt", bufs=2 * nkc) as ptp,
            tc.tile_pool(name="norm", bufs=2) as nmp,
            tc.tile_pool(name="tmp", bufs=4) as tmp,
            tc.tile_pool(name="outs", bufs=2) as otp,
            tc.tile_pool(name="psS", bufs=3, space="PSUM") as psS,
            tc.tile_pool(name="psC", bufs=4, space="PSUM") as psC,
            tc.tile_pool(name="psR", bufs=1, space="PSUM") as psR,
        ):
            # ---- constants (amortized across repeats) ----
            wq = []
            for t in range(KT):
                w1 = cp.tile([128, 384], f16, tag=f"wq{t}")
                nc.sync.dma_start(w1[:], Wqk[t * 128:(t + 1) * 128, :])
                wq.append(w1)
            pad = cp.tile([128, nkc], f32, tag="pad")
            nc.sync.dma_start(pad[:], padc)
            wv = []
            for t in range(KT):
                w3 = cp.tile([128, 65 * HPC], f16, tag=f"wv{t}")
                nc.sync.dma_start(w3[:], WvA[t * 128:(t + 1) * 128, :])
                wv.append(w3)
            o64 = cp.tile([1, DH], f16, tag="o64")
            nc.sync.dma_start(o64[:], ones64)
            idf = cp.tile([128, 128], fp8, tag="idf")
            nc.sync.dma_start(idf[:], idf8)
            wo01 = cp.tile([128, D], f16, tag="wo01")
            nc.sync.dma_start(wo01[:], WoR[0:2].rearrange("a b c -> (a b) c"))
            wo2 = cp.tile([DH, D], f16, tag="wo2")
            nc.sync.dma_start(wo2[:], WoR[2])

            def load_x(x_dram, nl, tag):
                xt_t = xtp.tile([128, KT * nl], f16, tag=tag)
                nc.sync.dma_start(xt_t[:], x_dram[:, 0:KT * nl])
                return xt_t

            def nchunks(n):
                return [(i, min(HF, n - i)) for i in range(0, n, HF)]

            # ---- q^T / k^T projection, one [rows, n] chunk, half-wise ----
            def proj_one(xbig, xs, col0, rows, n, tag, nl=None, eng=None):
                nl = n if nl is None else nl
                dst = qkp.tile([rows, n], f16, tag=tag)
                for n0, nw in nchunks(nl):
                    pps = psS.tile([128, HF], f32, tag="sps")
                    for t in range(KT):
                        nc.tensor.matmul(
                            pps[0:rows, 0:nw],
                            wq[t][:, col0: col0 + rows],
                            xbig[:, t * xs + n0:t * xs + n0 + nw],
                            start=(t == 0), stop=(t == KT - 1))
                    if eng == "dve":
                        nc.vector.tensor_copy(dst[:, n0:n0 + nw],
                                              pps[0:rows, 0:nw])
                    else:
                        nc.scalar.copy(dst[:, n0:n0 + nw], pps[0:rows, 0:nw])
                if nl < n:
                    nc.gpsimd.memset(dst[:, nl:n], 0.0)
                return dst

            def load_mask():
                mb = mkp.tile([128, max(nfull, 1) * HPC * SQ], fp8,
                              tag="mask")
                nc.sync.dma_start(mb[:, 0:nfull * HPC * SQ],
                                  maskT[:, 0:nfull * HPC * SQ])
                mt = None
                if r_t:
                    mt = mkp.tile([128, HPC * SQ], fp8, tag="mtail")
                    nc.gpsimd.memset(mt[:], 0.0)
                    nc.sync.dma_start(mt[0:r_t, :], maskU[0:r_t, :])
                return mb, mt

            def load_rep():
                qx = load_x(qT, SQ, "xq")
                kx = load_x(kT, n_trim, "xk")
                mbig, mtail = load_mask()
                vx = load_x(vT, n_trim, "xv")
                return qx, kx, vx, mbig, mtail

            def emit_rep(tiles):
                qx, kx, vx, mbig, mtail = tiles
                if stage == 0:
                    for t in range(8):
                        ot = otp.tile([128, D], f16, tag=f"ot{t % 3}")
                        nc.gpsimd.memset(ot[:], 0.0)
                        nc.sync.dma_start(out_d[t * 128:(t + 1) * 128, :],
                                          ot[:])
                    return
                q_c0 = proj_one(qx, SQ, 0, 128, SQ, "q0")
                q_c1 = proj_one(qx, SQ, 128, DH, SQ, "q1", eng="dve")
                k_c0 = proj_one(kx, n_trim, 192, 128, SK_P, "k0", n_trim)
                k_c1 = proj_one(kx, n_trim, 192 + 128, DH, SK_P, "k1",
                                n_trim, eng="dve")
                qh = [q_c0[0:DH, :], q_c0[DH:128, :], q_c1]
                kh = [k_c0[0:DH, :], k_c0[DH:128, :], k_c1]

                # ---- attention ----
                vtiles = []
                cn01 = nmp.tile([128, SQ], f16, tag="cn01")
                cn2 = nmp.tile([DH, SQ], f16, tag="cn2")
                ctxh = {}
                pts = {}

                def emit_tile(j, i):
                    msrc = mtail if i >= nfull else mbig
                    mof = 0 if i >= nfull else i * HPC * SQ
                    pt = ptp.tile([128, SQ], f16, tag="pt")
                    for n0, nw in nchunks(SQ):
                        sps = psS.tile([128, HF], f32, tag="sps")
                        nc.tensor.matmul(
                            sps[:, 0:nw],
                            kh[j][:, i * 128:(i + 1) * 128],
                            qh[j][:, n0:n0 + nw],
                            start=True, stop=False)
                        nc.tensor.matmul(
                            sps[:, 0:nw], idf[:],
                            msrc[:, mof + j * SQ + n0:mof + j * SQ + n0 + nw],
                            start=False, stop=True)
                        nc.scalar.activation(pt[:, n0:n0 + nw], sps[:, 0:nw],
                                             AF.Exp, bias=pad[:, i:i + 1],
                                             scale=1.0)
                    pts[(j, i)] = pt

                def emit_av(j, i):
                    pt_ = pts[(j, i)]
                    for hh, (n0, nw) in enumerate(nchunks(SQ)):
                        if (j, hh) not in ctxh:
                            ctxh[(j, hh)] = psC.tile([65, HF], f32, tag="cx",
                                                     name=f"cx{j}_{hh}")
                        nc.tensor.matmul(
                            ctxh[(j, hh)][:, 0:nw],
                            vtiles[i][:, j * 65:(j + 1) * 65],
                            pt_[:, n0:n0 + nw],
                            start=(i == 0), stop=(i == nkc - 1))
                    del pts[(j, i)]

                def emit_norm(j):
                    for hh, (n0, nw) in enumerate(nchunks(SQ)):
                        ctx = ctxh.pop((j, hh))
                        rd = tmp.tile([1, HF], f32, tag="rd")
                        nc.vector.reciprocal(rd[:, 0:nw],
                                             ctx[DH:DH + 1, 0:nw])
                        rdb = tmp.tile([DH, HF], f32, tag="rdb")
                        nc.gpsimd.partition_broadcast(rdb[:, 0:nw],
                                                      rd[:, 0:nw])
                        dst = (cn01[j * DH:(j + 1) * DH] if j < 2 else
                               cn2[:])
                        nc.vector.tensor_mul(dst[:, n0:n0 + nw],
                                             ctx[0:DH, 0:nw], rdb[:, 0:nw])

                def vproj_tile(i, vx):
                    rl = min(128, n_trim - i * 128)
                    vps = psR.tile([128, 65 * HPC], f32, tag="rr")
                    for t in range(KT):
                        nc.tensor.matmul(
                            vps[0:rl, :],
                            vx[:, t * n_trim + i * 128:
                               t * n_trim + i * 128 + rl],
                            wv[t][:],
                            start=(t == 0), stop=(t == KT - 1))
                    vt = vvp.tile([128, 65 * HPC], f16, tag=f"v{i}")
                    if rl < 128:
                        nc.gpsimd.memset(vt[:], 0.0)
                    nc.scalar.copy(vt[0:rl, :], vps[0:rl, :])
                    for j in range(HPC):
                        nc.gpsimd.memset(
                            vt[0:rl, j * 65 + DH:j * 65 + DH + 1], 1.0)
                    vtiles.append(vt)

                # head 0 logits chain (AVs deferred until v exists)
                for i in range(nkc):
                    emit_tile(0, i)

                # head 1 chain; per tile: v-proj(i) + AV(0,i) fused in
                for i in range(nkc):
                    vproj_tile(i, vx)
                    emit_av(0, i)
                    emit_tile(1, i)
                    if i > 0:
                        emit_av(1, i - 1)
                emit_norm(0)
                # head 2 chain
                for i in range(nkc):
                    emit_tile(2, i)
                    if i == 0:
                        emit_av(1, nkc - 1)
                    else:
                        emit_av(2, i - 1)
                emit_norm(1)
                emit_av(2, nkc - 1)
                emit_norm(2)

                # ---- output projection ----
                for t in range(8):
                    ot = otp.tile([128, D], f16, tag=f"ot{t % 3}")
                    for hh, (n0, nw) in enumerate(((0, 512), (512, 256))):
                        ops = psS.tile([128, HF], f32, tag="sps")
                        nc.tensor.matmul(
                            ops[:, 0:nw],
                            cn01[:, t * 128:(t + 1) * 128],
                            wo01[:, n0:n0 + nw],
                            start=True, stop=False)
                        nc.tensor.matmul(
                            ops[:, 0:nw],
                            cn2[:, t * 128:(t + 1) * 128],
                            wo2[:, n0:n0 + nw],
                            start=False, stop=True)
                        if (2 * t + hh) % 2 == 0:
                            nc.vector.tensor_copy(ot[:, n0:n0 + nw],
                                                  ops[:, 0:nw])
                        else:
                            nc.scalar.copy(ot[:, n0:n0 + nw], ops[:, 0:nw])
                    nc.gpsimd.dma_start(out_d[t * 128:(t + 1) * 128, :],
                                        ot[:])

            def emit_body():
                tiles = load_rep()
                for r in range(repeats):
                    nxt = load_rep() if r + 1 < repeats else None
                    emit_rep(tiles)
                    tiles = nxt

            if loop_n:
                ET = mybir.EngineType
                with tc.For_i(0, loop_n, 1,
                              hint_engines=(ET.PE, ET.DVE, ET.Activation,
                                            ET.Pool, ET.SP)):
                    emit_body()
            else:
                emit_body()

    nc.compile()
    return nc


def prep_inputs(value, key, query, key_padding_mask, attn_mask,
                Wq, Wk, Wv, Wo, bq, bk, bv, bo):
    import ml_dtypes

    f = np.float32
    h = np.float16
    value = np.asarray(value, f)
    key = np.asarray(key, f)
    query = np.asarray(query, f)
    key_padding_mask = np.asarray(key_padding_mask)
    attn_mask = np.asarray(attn_mask, f)
    Wq, Wk, Wv, Wo = (np.asarray(w, f) for w in (Wq, Wk, Wv, Wo))
    for b_ in (bq, bk, bv):
        assert np.abs(np.asarray(b_)).max() == 0.0, \
            "nonzero qkv biases unsupported in this build"

    scale = f(1.0 / np.sqrt(DH))
    idx = [np.nonzero(key_padding_mask[b])[0] for b in range(B)]
    n_trim = max(1, max(len(ix) for ix in idx))
    nkc = -(-n_trim // 128)
    SK_P = nkc * 128

    xT = {}
    padcs = {}
    for b in range(B):
        ix = idx[b]
        n = len(ix)
        kc = np.zeros((SK_P, D), f)
        vc = np.zeros((SK_P, D), f)
        kc[:n] = key[b][ix]
        vc[:n] = value[b][ix]
        def pack(m, nl):
            # [768, nl] -> [128, KT*nl], block t = rows t*128:(t+1)*128
            mm = m.T.astype(h)[:, :nl]
            return np.ascontiguousarray(
                mm.reshape(KT, 128, nl).transpose(1, 0, 2).reshape(
                    128, KT * nl))
        xT[("q", b)] = pack(query[b], SQ)
        xT[("k", b)] = pack(kc, n_trim)
        xT[("v", b)] = pack(vc, n_trim)
        pc = np.full((SK_P,), ESH, f)
        pc[n:] = NEG
        padcs[b] = np.ascontiguousarray(pc.reshape(nkc, 128).T)

    in_maps = []
    for c in range(N_CORES):
        b, g = divmod(c, GPB)
        h0 = g * HPC
        cols = slice(h0 * DH, (h0 + HPC) * DH)
        WqkA = np.zeros((D, 384), f)
        WqkA[:, 0:192] = Wq[:, cols] * scale
        WqkA[:, 192:384] = Wk[:, cols]
        WvC = np.zeros((D, 65 * HPC), f)
        for j in range(HPC):
            hc = slice((h0 + j) * DH, (h0 + j + 1) * DH)
            WvC[:, j * 65:j * 65 + DH] = Wv[:, hc]
        WoR = np.ascontiguousarray(Wo[cols].reshape(HPC, DH, D)).astype(h)
        ix = idx[b]
        mk = np.zeros((SK_P, HPC * SQ), f)
        for j in range(HPC):
            mk[:len(ix), j * SQ:(j + 1) * SQ] = attn_mask[b, h0 + j].T[ix]
        mk8 = mk.astype(ml_dtypes.float8_e4m3fn)
        nfull = n_trim // 128
        r_t = n_trim - 128 * nfull
        mbig = np.zeros((128, max(nfull, 1) * HPC * SQ),
                        ml_dtypes.float8_e4m3fn)
        for i in range(nfull):
            mbig[:, i * HPC * SQ:(i + 1) * HPC * SQ] = \
                mk8[i * 128:(i + 1) * 128]
        mtail = np.zeros((128, HPC * SQ), ml_dtypes.float8_e4m3fn)
        if r_t:
            mtail[0:r_t] = mk8[nfull * 128:nfull * 128 + r_t]
        in_maps.append({
            "qT": xT[("q", b)],
            "kT": xT[("k", b)],
            "vT": xT[("v", b)],
            "Wqk": WqkA.astype(h),
            "WvA": WvC.astype(h),
            "WoR": WoR,
            "maskT": mbig,
            "maskU": mtail,
            "padc": padcs[b],
            "ones64": np.ones((1, DH), h),
            "idf8": np.eye(128).astype(ml_dtypes.float8_e4m3fn),
        })
    return in_maps, nkc, n_trim


def get_nc(nkc, n_trim, repeats=1, stage=3, loop_n=0):
    key = ("nc", nkc, n_trim, repeats, stage, loop_n)
    if key not in _CACHE:
        _CACHE[key] = _build(nkc, n_trim, repeats, stage, loop_n)
    return _CACHE[key]


def assemble(results, bo):
    out = np.zeros((B, SQ, D), np.float32)
    for c in range(N_CORES):
        out[c // GPB] += results[c]["out"].astype(np.float32)
    return out + np.asarray(bo, np.float32)


def bench_prep(inputs):
    import os
    in_maps, nkc, n_trim = prep_inputs(**inputs)
    stage = int(os.environ.get("STAGE", "3"))
    return in_maps, (lambda reps=1, loop_n=0: get_nc(nkc, n_trim, reps,
                                                     stage, loop_n))


def kernel(value, key, query, key_padding_mask, attn_mask,
           Wq, Wk, Wv, Wo, bq, bk, bv, bo, **extra):
    from concourse.bass_utils import run_bass_kernel_spmd

    in_maps, nkc, n_trim = prep_inputs(value, key, query, key_padding_mask,
                                       attn_mask, Wq, Wk, Wv, Wo,
                                       bq, bk, bv, bo)
    nc = get_nc(nkc, n_trim)
    res = run_bass_kernel_spmd(nc, in_maps, core_ids=list(range(N_CORES)))
    _CACHE["last_results"] = res
    return assemble(res.results, bo)


# revision 5
# speedup vs baseline: 1.2167x; 1.1092x over previous
"""Multi-head attention (B=2, S=1024, D=768, H=12) on 8 TRN2 NeuronCores. v13.

Sharding 2x2: core c handles batch b = c//4, head-group hg = (c%4)//2
(6 heads), query-half qh = c%2 (512 queries). Each core computes q/k/v
projections for its 6 heads (q only for its query half), attention, and a
partial output projection; the host sums the 2 head-group partials per
(batch, query-half). Versus batch x 4-head-groups this halves the
query-input duplication and the output-partial bytes (~1.5 MB/core less
HBM traffic) and makes every projection/outproj chain a full 128-column
stationary (no 64-row half-efficiency chains), at the cost of doubled
k/v projection work - cheap, since PE measures ~87 ns per 512-wide MM.

Everything else as v9: key compaction to the exact valid-key count,
fp16 inputs/weights/intermediates, fp8 e4m3 mask accumulated into the
logits PSUM by the PE via an identity matmul, exp on ACT from PSUM with
the key-padding/-8 bias, denominators via a ones-column of v, norm via
DVE reciprocal + GPSIMD partition_broadcast, POOL-issued output DMAs,
software-pipelined loads across bench repeats.
"""

import numpy as np

B, SQ, D, H = 2, 1024, 768, 12
DH = D // H            # 64
HPC = 6                # heads per core
SQL = 512              # queries per core
N_CORES = 8
KT = 6                 # k-tiles over the contraction dim 768
NEG = -1.0e30
ESH = -8.0             # exp shift

_CACHE = {}


def _build(nkc, n_trim, repeats=1, stage=3, loop_n=0):
    import concourse.tile as tile
    import concourse.mybir as mybir
    from concourse import bacc

    SK_P = nkc * 128
    assert 0 < n_trim <= SK_P
    nfull = n_trim // 128
    r_t = n_trim - 128 * nfull
    f32 = mybir.dt.float32
    f16 = mybir.dt.float16
    fp8 = mybir.dt.float8e4
    AF = mybir.ActivationFunctionType

    nc = bacc.Bacc("TRN2", target_bir_lowering=False, debug=False,
                   num_devices=N_CORES)

    qT = nc.dram_tensor("qT", [128, KT * SQL], f16, kind="ExternalInput").ap()
    kT = nc.dram_tensor("kT", [128, KT * n_trim], f16,
                        kind="ExternalInput").ap()
    vT = nc.dram_tensor("vT", [128, KT * n_trim], f16,
                        kind="ExternalInput").ap()
    # Wqk: cols 0:384 = Wq 6 heads (pre-scaled), cols 384:768 = Wk
    Wqk = nc.dram_tensor("Wqk", [D, 768], f16, kind="ExternalInput").ap()
    WvA = nc.dram_tensor("WvA", [D, 65 * HPC], f16, kind="ExternalInput").ap()
    WoR = nc.dram_tensor("WoR", [HPC, DH, D], f16, kind="ExternalInput").ap()
    padc = nc.dram_tensor("padc", [128, nkc], f32, kind="ExternalInput").ap()
    idf8 = nc.dram_tensor("idf8", [128, 128], fp8, kind="ExternalInput").ap()
    maskT = nc.dram_tensor("maskT", [128, max(nfull, 1) * HPC * SQL], fp8,
                           kind="ExternalInput").ap()
    maskU = nc.dram_tensor("maskU", [128, HPC * SQL], fp8,
                           kind="ExternalInput").ap()
    out_d = nc.dram_tensor("out", [SQL, D], f16, kind="ExternalOutput").ap()

    with tile.TileContext(nc) as tc:
        with (
            tc.tile_pool(name="consts", bufs=1) as cp,
            tc.tile_pool(name="xt", bufs=2) as xtp,
            tc.tile_pool(name="qk", bufs=2) as qkp,
            tc.tile_pool(name="vv", bufs=2) as vvp,
            tc.tile_pool(name="mask", bufs=2) as mkp,
            tc.tile_pool(name="pt", bufs=2 * nkc) as ptp,
            tc.tile_pool(name="norm", bufs=2) as nmp,
            tc.tile_pool(name="tmp", bufs=4) as tmp,
            tc.tile_pool(name="outs", bufs=2) as otp,
            tc.tile_pool(name="psS", bufs=3, space="PSUM") as psS,
            tc.tile_pool(name="psC", bufs=3, space="PSUM") as psC,
            tc.tile_pool(name="psR", bufs=2, space="PSUM") as psR,
        ):
            # ---- constants (amortized across repeats) ----
            wq = []
            for t in range(KT):
                w1 = cp.tile([128, 768], f16, tag=f"wq{t}")
                nc.sync.dma_start(w1[:], Wqk[t * 128:(t + 1) * 128, :])
                wq.append(w1)
            pad = cp.tile([128, nkc], f32, tag="pad")
            nc.sync.dma_start(pad[:], padc)
            wv = []
            for t in range(KT):
                w3 = cp.tile([128, 65 * HPC], f16, tag=f"wv{t}")
                nc.sync.dma_start(w3[:], WvA[t * 128:(t + 1) * 128, :])
                wv.append(w3)
            idf = cp.tile([128, 128], fp8, tag="idf")
            nc.sync.dma_start(idf[:], idf8)
            wo = []
            for g in range(3):
                wg = cp.tile([128, D], f16, tag=f"wo{g}")
                nc.sync.dma_start(
                    wg[:], WoR[2 * g:2 * g + 2].rearrange("a b c -> (a b) c"))
                wo.append(wg)

            def load_x(x_dram, nl, tag):
                xt_t = xtp.tile([128, KT * nl], f16, tag=tag)
                nc.sync.dma_start(xt_t[:], x_dram[:, 0:KT * nl])
                return xt_t

            def load_mask():
                mb = mkp.tile([128, max(nfull, 1) * HPC * SQL], fp8,
                              tag="mask")
                nc.sync.dma_start(mb[:, 0:nfull * HPC * SQL],
                                  maskT[:, 0:nfull * HPC * SQL])
                mt = None
                if r_t:
                    mt = mkp.tile([128, HPC * SQL], fp8, tag="mtail")
                    nc.gpsimd.memset(mt[:], 0.0)
                    nc.sync.dma_start(mt[0:r_t, :], maskU[0:r_t, :])
                return mb, mt

            def load_rep():
                qx = load_x(qT, SQL, "xq")
                kx = load_x(kT, n_trim, "xk")
                mbig, mtail = load_mask()
                vx = load_x(vT, n_trim, "xv")
                return qx, kx, vx, mbig, mtail

            # one [128-rows, n] projection chunk from a packed x big-tile
            def proj_one(xbig, xs, col0, n, tag, nl=None, eng=None):
                nl = n if nl is None else nl
                dst = qkp.tile([128, n], f16, tag=tag)
                for n0 in range(0, nl, 512):
                    nw = min(512, nl - n0)
                    pps = psS.tile([128, 512], f32, tag="sps")
                    for t in range(KT):
                        nc.tensor.matmul(
                            pps[:, 0:nw],
                            wq[t][:, col0: col0 + 128],
                            xbig[:, t * xs + n0:t * xs + n0 + nw],
                            start=(t == 0), stop=(t == KT - 1))
                    if eng == "dve":
                        nc.vector.tensor_copy(dst[:, n0:n0 + nw],
                                              pps[:, 0:nw])
                    else:
                        nc.scalar.copy(dst[:, n0:n0 + nw], pps[:, 0:nw])
                if nl < n:
                    nc.gpsimd.memset(dst[:, nl:n], 0.0)
                return dst

            def emit_rep(tiles):
                qx, kx, vx, mbig, mtail = tiles
                if stage == 0:
                    for t in range(4):
                        ot = otp.tile([128, D], f16, tag=f"ot{t % 2}")
                        nc.gpsimd.memset(ot[:], 0.0)
                        nc.sync.dma_start(out_d[t * 128:(t + 1) * 128, :],
                                          ot[:])
                    return
                qc = [proj_one(qx, SQL, g * 128, SQL, f"q{g}",
                               eng=("dve" if g % 2 else None))
                      for g in range(3)]
                kc = [proj_one(kx, n_trim, 384 + g * 128, SK_P, f"k{g}",
                               n_trim, eng=("dve" if g % 2 == 0 else None))
                      for g in range(3)]

                def qs(j):
                    return qc[j // 2][(j % 2) * DH:(j % 2 + 1) * DH, :]

                def ks(j):
                    return kc[j // 2][(j % 2) * DH:(j % 2 + 1) * DH, :]

                # ---- attention ----
                vtiles = []
                cn = [nmp.tile([128, SQL], f16, tag=f"cn{g}", name=f"cn{g}")
                      for g in range(3)]
                ctxs = {}
                pts = {}

                def emit_tile(j, i):
                    msrc = mtail if i >= nfull else mbig
                    mof = (0 if i >= nfull else i * HPC * SQL) + j * SQL
                    pt = ptp.tile([128, SQL], f16, tag="pt")
                    sps = psS.tile([128, 512], f32, tag="sps")
                    nc.tensor.matmul(
                        sps[:], ks(j)[:, i * 128:(i + 1) * 128], qs(j)[:],
                        start=True, stop=False)
                    nc.tensor.matmul(
                        sps[:], idf[:], msrc[:, mof:mof + SQL],
                        start=False, stop=True)
                    nc.scalar.activation(pt[:], sps[:], AF.Exp,
                                         bias=pad[:, i:i + 1], scale=1.0)
                    pts[(j, i)] = pt

                def emit_av(j, i):
                    if j not in ctxs:
                        ctxs[j] = psC.tile([65, SQL], f32, tag="cx",
                                           name=f"cx{j}")
                    nc.tensor.matmul(
                        ctxs[j][:], vtiles[i][:, j * 65:(j + 1) * 65],
                        pts[(j, i)][:],
                        start=(i == 0), stop=(i == nkc - 1))
                    del pts[(j, i)]

                def emit_norm(j):
                    ctx = ctxs.pop(j)
                    rd = tmp.tile([1, SQL], f32, tag="rd")
                    nc.vector.reciprocal(rd[:], ctx[DH:DH + 1, :])
                    rdb = tmp.tile([DH, SQL], f32, tag="rdb")
                    nc.gpsimd.partition_broadcast(rdb[:], rd[:])
                    nc.vector.tensor_mul(
                        cn[j // 2][(j % 2) * DH:(j % 2 + 1) * DH, :],
                        ctx[0:DH, :], rdb[:])

                def vproj_tile(i, vx):
                    rl = min(128, n_trim - i * 128)
                    vps = psR.tile([128, 65 * HPC], f32, tag="rr")
                    for t in range(KT):
                        nc.tensor.matmul(
                            vps[0:rl, :],
                            vx[:, t * n_trim + i * 128:
                               t * n_trim + i * 128 + rl],
                            wv[t][:],
                            start=(t == 0), stop=(t == KT - 1))
                    vt = vvp.tile([128, 65 * HPC], f16, tag=f"v{i}")
                    if rl < 128:
                        nc.gpsimd.memset(vt[:], 0.0)
                    nc.scalar.copy(vt[0:rl, :], vps[0:rl, :])
                    for j in range(HPC):
                        nc.gpsimd.memset(
                            vt[0:rl, j * 65 + DH:j * 65 + DH + 1], 1.0)
                    vtiles.append(vt)

                # head 0 chain (AVs deferred until v exists)
                for i in range(nkc):
                    emit_tile(0, i)
                # head 1 chain with v-projection and AV(0) fused in
                for i in range(nkc):
                    vproj_tile(i, vx)
                    emit_av(0, i)
                    emit_tile(1, i)
                    if i > 0:
                        emit_av(1, i - 1)
                emit_norm(0)
                # heads 2..5
                for j in range(2, HPC):
                    for i in range(nkc):
                        emit_tile(j, i)
                        if i == 0:
                            emit_av(j - 1, nkc - 1)
                        else:
                            emit_av(j, i - 1)
                    emit_norm(j - 1)
                emit_av(HPC - 1, nkc - 1)
                emit_norm(HPC - 1)

                # ---- output projection ----
                for t in range(4):
                    ot = otp.tile([128, D], f16, tag=f"ot{t % 2}")
                    for hh, (n0, nw) in enumerate(((0, 512), (512, 256))):
                        ops = psS.tile([128, 512], f32, tag="sps")
                        for g in range(3):
                            nc.tensor.matmul(
                                ops[:, 0:nw],
                                cn[g][:, t * 128:(t + 1) * 128],
                                wo[g][:, n0:n0 + nw],
                                start=(g == 0), stop=(g == 2))
                        if hh == 1:
                            nc.vector.tensor_copy(ot[:, n0:n0 + nw],
                                                  ops[:, 0:nw])
                        else:
                            nc.scalar.copy(ot[:, n0:n0 + nw], ops[:, 0:nw])
                    nc.gpsimd.dma_start(out_d[t * 128:(t + 1) * 128, :],
                                        ot[:])

            def emit_body():
                tiles = load_rep()
                for r in range(repeats):
                    nxt = load_rep() if r + 1 < repeats else None
                    emit_rep(tiles)
                    tiles = nxt

            if loop_n:
                ET = mybir.EngineType
                with tc.For_i(0, loop_n, 1,
                              hint_engines=(ET.PE, ET.DVE, ET.Activation,
                                            ET.Pool, ET.SP)):
                    emit_body()
            else:
                emit_body()

    nc.compile()
    return nc


def prep_inputs(value, key, query, key_padding_mask, attn_mask,
                Wq, Wk, Wv, Wo, bq, bk, bv, bo):
    import ml_dtypes

    f = np.float32
    h = np.float16
    value = np.asarray(value, f)
    key = np.asarray(key, f)
    query = np.asarray(query, f)
    key_padding_mask = np.asarray(key_padding_mask)
    attn_mask = np.asarray(attn_mask, f)
    Wq, Wk, Wv, Wo = (np.asarray(w, f) for w in (Wq, Wk, Wv, Wo))
    for b_ in (bq, bk, bv):
        assert np.abs(np.asarray(b_)).max() == 0.0, \
            "nonzero qkv biases unsupported in this build"

    scale = f(1.0 / np.sqrt(DH))
    idx = [np.nonzero(key_padding_mask[b])[0] for b in range(B)]
    n_trim = max(1, max(len(ix) for ix in idx))
    nkc = -(-n_trim // 128)
    SK_P = nkc * 128
    nfull = n_trim // 128
    r_t = n_trim - 128 * nfull

    def pack(m, nl):
        # [rows, 768] -> [128, KT*nl], block t = dims t*128:(t+1)*128
        mm = m.T.astype(h)[:, :nl]
        return np.ascontiguousarray(
            mm.reshape(KT, 128, nl).transpose(1, 0, 2).reshape(128, KT * nl))

    kv = {}
    padcs = {}
    for b in range(B):
        ix = idx[b]
        n = len(ix)
        kc = np.zeros((SK_P, D), f)
        vc = np.zeros((SK_P, D), f)
        kc[:n] = key[b][ix]
        vc[:n] = value[b][ix]
        kv[("k", b)] = pack(kc, n_trim)
        kv[("v", b)] = pack(vc, n_trim)
        pc = np.full((SK_P,), ESH, f)
        pc[n:] = NEG
        padcs[b] = np.ascontiguousarray(pc.reshape(nkc, 128).T)

    in_maps = []
    for c in range(N_CORES):
        b = c // 4
        g4 = c % 4
        hg, qh = divmod(g4, 2)
        h0 = hg * HPC
        cols = slice(h0 * DH, (h0 + HPC) * DH)
        WqkA = np.zeros((D, 768), f)
        WqkA[:, 0:384] = Wq[:, cols] * scale
        WqkA[:, 384:768] = Wk[:, cols]
        WvC = np.zeros((D, 65 * HPC), f)
        for j in range(HPC):
            hc = slice((h0 + j) * DH, (h0 + j + 1) * DH)
            WvC[:, j * 65:j * 65 + DH] = Wv[:, hc]
        WoR = np.ascontiguousarray(Wo[cols].reshape(HPC, DH, D)).astype(h)
        ix = idx[b]
        qsl = slice(qh * SQL, (qh + 1) * SQL)
        mk = np.zeros((SK_P, HPC * SQL), f)
        for j in range(HPC):
            mk[:len(ix), j * SQL:(j + 1) * SQL] = \
                attn_mask[b, h0 + j].T[ix][:, qsl]
        mk8 = mk.astype(ml_dtypes.float8_e4m3fn)
        mbig = np.zeros((128, max(nfull, 1) * HPC * SQL),
                        ml_dtypes.float8_e4m3fn)
        for i in range(nfull):
            mbig[:, i * HPC * SQL:(i + 1) * HPC * SQL] = \
                mk8[i * 128:(i + 1) * 128]
        mtail = np.zeros((128, HPC * SQL), ml_dtypes.float8_e4m3fn)
        if r_t:
            mtail[0:r_t] = mk8[nfull * 128:nfull * 128 + r_t]
        in_maps.append({
            "qT": pack(query[b][qsl], SQL),
            "kT": kv[("k", b)],
            "vT": kv[("v", b)],
            "Wqk": WqkA.astype(h),
            "WvA": WvC.astype(h),
            "WoR": WoR,
            "maskT": mbig,
            "maskU": mtail,
            "padc": padcs[b],
            "idf8": np.eye(128).astype(ml_dtypes.float8_e4m3fn),
        })
    return in_maps, nkc, n_trim


def get_nc(nkc, n_trim, repeats=1, stage=3, loop_n=0):
    key = ("nc", nkc, n_trim, repeats, stage, loop_n)
    if key not in _CACHE:
        _CACHE[key] = _build(nkc, n_trim, repeats, stage, loop_n)
    return _CACHE[key]


def assemble(results, bo):
    out = np.zeros((B, SQ, D), np.float32)
    for c in range(N_CORES):
        b = c // 4
        qh = (c % 4) % 2
        out[b, qh * SQL:(qh + 1) * SQL] += results[c]["out"].astype(np.float32)
    return out + np.asarray(bo, np.float32)


def bench_prep(inputs):
    import os
    in_maps, nkc, n_trim = prep_inputs(**inputs)
    stage = int(os.environ.get("STAGE", "3"))
    return in_maps, (lambda reps=1, loop_n=0: get_nc(nkc, n_trim, reps,
                                                     stage, loop_n))


def kernel(value, key, query, key_padding_mask, attn_mask,
           Wq, Wk, Wv, Wo, bq, bk, bv, bo, **extra):
    from concourse.bass_utils import run_bass_kernel_spmd

    in_maps, nkc, n_trim = prep_inputs(value, key, query, key_padding_mask,
                                       attn_mask, Wq, Wk, Wv, Wo,
                                       bq, bk, bv, bo)
    nc = get_nc(nkc, n_trim)
    res = run_bass_kernel_spmd(nc, in_maps, core_ids=list(range(N_CORES)))
    _CACHE["last_results"] = res
    return assemble(res.results, bo)
